# revision 21
# baseline (speedup 1.0000x reference)
"""AtomPosGNN distributed Trainium2 kernel (8 NeuronCores).

Reference computation (N=8192 nodes, H=128 features, L=4 layers):
    feat = concat(atom_pos, atom_emb)            # [N, 128]
    deg = dist_adj.sum(-1); isd = rsqrt(deg)
    for l in range(4):
        h = (feat * isd[:, None]) @ Ws[l]
        h = dist_adj @ h
        feat = softplus(h * isd[:, None] + bs[l])

Strategy (row shard, P=1024 rows per core, transpose-free):
  - Host passes each core its adj row-block ALREADY transposed and cast
    to bf16: adjT_c = dist_adj[rows_c, :].T  ([N, P] bf16, node index on
    partitions after tiling). No device-side transposes.
  - Prep: warm-AG trigger at t=0 on gpsimd (nothing else on that
    engine first), adjT streamed on the sync+scalar HWDGE queues; deg =
    ones-stationary matmul pass riding behind the DMA; isd via PE
    ones-broadcast + full-width reciprocal/sqrt.
  - Per layer the 1024 output columns run in 3 telescoped passes
    (512/256/256 cols). After each pass its epilogue + local g +
    AllGather (128/64/64KB) overlap the following passes; the next
    layer consumes kb tiles in matching group order (k0-3, k4-5, k6-7
    within each rank block, k-major) so only the small last AG is ever
    near the critical path.
  - adj is read from HBM exactly once; layers run entirely from SBUF.
"""

import os
import sys

for _p in ("/opt/trn_rl_repo",):
    if _p not in sys.path and os.path.isdir(_p):
        sys.path.insert(0, _p)

import numpy as np
import ml_dtypes

import concourse.bacc as bacc
import concourse.bass as bass
import concourse.mybir as mybir
import concourse.tile as tile
from concourse.bass_utils import run_bass_kernel_spmd

R = 8          # cores
N = 8192       # nodes
P = N // R     # local rows = 1024
H = 128        # hidden
L = 4          # layers
KB = N // 128  # 64 k-tiles
KBD = 4        # k-tiles per adj dma_start

F32 = mybir.dt.float32
BF16 = mybir.dt.bfloat16

WARM_AG = os.environ.get("K_WARM", "1") == "1"

LOG_A = float(np.log(2.0) / (1 << 23))
LOG_B = float(-np.log(2.0) * (127 + 0.0450466))

LAST_RESULT = None
_NC_CACHE = {}

# row/column halves: (lo, hi) in local node index
GROUPS = [(0, 512), (512, 1024)]
# kb consumption order: group-major, then k-major within group, then rank
KB_ORDER = []
for lo, hi in GROUPS:
    for k in range(lo // 128, hi // 128):
        for r in range(R):
            KB_ORDER.append((k, r))
N_FILL = int(os.environ.get("K_FILL", "0"))  # keep-warm MMs (Tile reorders them; off)
PE_DEG_KB = 16  # deg k-tiles on the PE; the rest accumulate on the DVE


def _softplus(nc, sp_pool, out_ap, in_ap, bias_ap, hw):
    """out = softplus(in + bias), composed (no Softplus/Ln table on HW)."""
    z0 = sp_pool.tile([H, hw], F32, name="z0", tag=f"sp_b{hw}")
    nc.scalar.activation(
        z0[:, :], in_ap, mybir.ActivationFunctionType.Exp, bias=bias_ap, scale=1.0
    )
    z = sp_pool.tile([H, hw], F32, name="z", tag=f"sp_c{hw}")
    nc.vector.tensor_scalar_add(z[:, :], z0[:, :], 1.0)
    y0 = sp_pool.tile([H, hw], F32, name="y0", tag=f"sp_d{hw}")
    nc.vector.tensor_scalar(
        y0[:, :], z[:, :].bitcast(mybir.dt.int32), LOG_A, LOG_B,
        mybir.AluOpType.mult, mybir.AluOpType.add,
    )
    w_e = sp_pool.tile([H, hw], F32, name="w_e", tag=f"sp_e{hw}")
    nc.scalar.activation(
        w_e[:, :], y0[:, :], mybir.ActivationFunctionType.Exp, scale=-1.0
    )
    t1 = sp_pool.tile([H, hw], F32, name="t1", tag=f"sp_f{hw}")
    nc.vector.tensor_tensor(t1[:, :], z[:, :], w_e[:, :], mybir.AluOpType.mult)
    nc.vector.tensor_scalar_add(t1[:, :], t1[:, :], -1.0)
    nc.vector.tensor_tensor(out_ap, t1[:, :], y0[:, :], mybir.AluOpType.add)


def build_nc():
    nc = bacc.Bacc("TRN2", target_bir_lowering=False, debug=False, num_devices=R)

    adjT_ext = nc.declare_dram_parameter("adjT", [N, P], BF16, isOutput=False)
    featT_ext = nc.declare_dram_parameter("featT", [H, P], F32, isOutput=False)
    ws_ext = nc.declare_dram_parameter("ws", [L, H, H], BF16, isOutput=False)
    bsT_ext = nc.declare_dram_parameter("bsT", [H, L], F32, isOutput=False)
    out_ext = nc.declare_dram_parameter("out", [H, P], F32, isOutput=True)

    with tile.TileContext(nc) as tc:
        with (
            tc.tile_pool(name="big", bufs=1) as big,
            tc.tile_pool(name="sb", bufs=1) as sb,
            tc.tile_pool(name="ftl", bufs=2) as ftl_pool,
            tc.tile_pool(name="pre", bufs=2) as pre_pool,
            tc.tile_pool(name="sp", bufs=1) as sp_pool,
            tc.tile_pool(name="gsb", bufs=2) as gsb_pool,
            tc.tile_pool(name="psd", bufs=1, space="PSUM") as psd,
            tc.tile_pool(name="psg", bufs=2, space="PSUM") as psg,
            tc.tile_pool(name="psy", bufs=1, space="PSUM") as psy,
            tc.tile_pool(name="dram", bufs=1, space="DRAM") as dram,
        ):
            # warm the collective path at the very start: the first
            # collective pays a large cold staging cost.
            if WARM_AG:
                warm_in = dram.tile([P, H], BF16, name="warm_in")
                warm_out = dram.tile([N, H], BF16, addr_space="Shared", name="warm_out")
                nc.gpsimd.collective_compute(
                    "AllGather",
                    mybir.AluOpType.bypass,
                    replica_groups=[list(range(R))],
                    ins=[warm_in[:, :]],
                    outs=[warm_out[:, :]],
                )

            # ---- persistent SBUF ----
            at = big.tile([128, KB, P], BF16, name="at")  # adjT resident
            ones = sb.tile([128, 1], BF16, name="ones")
            nc.vector.memset(ones[:, :], 1.0)
            ones_row = sb.tile([1, 128], F32, name="ones_row")
            nc.vector.memset(ones_row[:, :], 1.0)
            w_sb = sb.tile([128, L, H], BF16, name="w_sb")
            nc.sync.dma_start(
                out=w_sb[:, :, :], in_=ws_ext.rearrange("l k h -> k l h")
            )
            bsT_sb = sb.tile([H, L], F32, name="bsT_sb")
            nc.sync.dma_start(out=bsT_sb[:, :], in_=bsT_ext[:, :])
            ftl = ftl_pool.tile([H, P], F32, name="ftl", tag="ftl")
            nc.sync.dma_start(out=ftl[:, :], in_=featT_ext[:, :])

            # ---- prep: load adjT on the two HWDGE queues ----
            # small leading chunks so the deg pass can start early
            adjT_r = adjT_ext.rearrange("(kb p) r -> p kb r", p=128)
            qeng = [nc.sync, nc.scalar]
            chunks = []
            kb0 = 0
            while kb0 < KB:
                w = 2 if kb0 < 8 else KBD
                chunks.append((kb0, w))
                kb0 += w
            for ci, (kb0, w) in enumerate(chunks):
                qeng[ci % 2].dma_start(
                    out=at[:, kb0 : kb0 + w, :],
                    in_=adjT_r[:, kb0 : kb0 + w, :],
                )

            # deg[r] = sum_j adjT[j, r]: first PE_DEG_KB k-tiles on the PE
            # (they arrive first); the rest accumulate on the idle DVE and
            # fold into the same PSUM groups via two ones-matmuls.
            ones_f = sb.tile([128, 1], F32, name="ones_f")
            nc.vector.memset(ones_f[:, :], 1.0)
            deg_ps = psd.tile([1, 2, 512], F32, name="deg_ps")
            for kb in range(PE_DEG_KB):
                for hh in range(2):
                    nc.tensor.matmul(
                        deg_ps[:, hh, :],
                        ones[:, :],
                        at[:, kb, hh * 512 : (hh + 1) * 512],
                        start=(kb == 0),
                        stop=False,
                    )
            dacc = sb.tile([128, P], F32, name="dacc")
            nc.vector.memset(dacc[:, :], 0.0)
            for kb in range(PE_DEG_KB, KB):
                nc.vector.tensor_tensor(
                    dacc[:, :], dacc[:, :], at[:, kb, :], mybir.AluOpType.add
                )
            for hh in range(2):
                nc.tensor.matmul(
                    deg_ps[:, hh, :],
                    ones_f[:, :],
                    dacc[:, hh * 512 : (hh + 1) * 512],
                    start=False,
                    stop=True,
                )
            deg_row = sb.tile([1, P], F32, name="deg_row")
            nc.vector.tensor_copy(deg_row[0:1, :], deg_ps.rearrange("o h x -> o (h x)"))
            dbc_ps = psd.tile([128, 2, 512], F32, name="dbc_ps")
            for hh in range(2):
                nc.tensor.matmul(
                    dbc_ps[:, hh, :],
                    ones_row[:, :],
                    deg_row[0:1, hh * 512 : (hh + 1) * 512],
                    start=True,
                    stop=True,
                )
            isd_rep = sb.tile([128, P], F32, name="isd_rep")
            nc.vector.reciprocal(isd_rep[:, :], dbc_ps.rearrange("p h x -> p (h x)"))
            nc.scalar.sqrt(isd_rep[:, :], isd_rep[:, :])

            def make_g(l, gi, ftl_src):
                """Local g rows [lo, hi) -> DRAM, AllGather; returns AG out.

                Processed in 256-row chunks so the trigger chain pipelines
                across DVE/PE/sync before the collective fires."""
                lo, hi = GROUPS[gi]
                kw = (hi - lo) // 128
                g_in = dram.tile([hi - lo, H], BF16, name=f"g_in{l}_{gi}")
                g_in_r = g_in.rearrange("(k p) f -> p k f", p=128)
                for c0 in range(0, kw, 2):
                    clo = lo + c0 * 128
                    ftl_s = pre_pool.tile(
                        [H, 256], BF16, name="ftl_s", tag=f"ftls{gi}_{c0}"
                    )
                    nc.vector.tensor_tensor(
                        ftl_s[:, :], ftl_src[:, clo : clo + 256],
                        isd_rep[:, clo : clo + 256], mybir.AluOpType.mult,
                    )
                    g_ps = psg.tile(
                        [128, 2, H], F32, name="g_ps", tag="gps"
                    )
                    for nb in range(2):
                        nc.tensor.matmul(
                            g_ps[:, nb, :],
                            ftl_s[:, nb * 128 : (nb + 1) * 128],
                            w_sb[:, l, :],
                            start=True,
                            stop=True,
                        )
                    g_stage = pre_pool.tile(
                        [128, 2, H], BF16, name="g_stage", tag=f"gstage{gi}_{c0}"
                    )
                    nc.vector.tensor_copy(g_stage[:, :, :], g_ps[:, :, :])
                    nc.sync.dma_start(
                        out=g_in_r[:, c0 : c0 + 2, :],
                        in_=g_stage[:, :, :],
                    )
                g_out = dram.tile(
                    [R * (hi - lo), H], BF16, addr_space="Shared",
                    name=f"g_out{l}_{gi}",
                )
                nc.gpsimd.collective_compute(
                    "AllGather",
                    mybir.AluOpType.bypass,
                    replica_groups=[list(range(R))],
                    ins=[g_in[:, :]],
                    outs=[g_out[:, :]],
                )
                return g_out

            def load_g(g_sb, gi, g_out, eng):
                """One DMA per k-tile of the group into k-major g_sb."""
                lo, hi = GROUPS[gi]
                kw = (hi - lo) // 128
                g_out_r = g_out.rearrange("(r k p) f -> p k r f", p=128, k=kw)
                for k in range(kw):
                    eng.dma_start(
                        out=g_sb[:, lo // 128 + k, :, :],
                        in_=g_out_r[:, k, :, :],
                    )

            def fillers(n, seed):
                """Keep-warm MMs into the dead deg bank: hold the PE HAM
                at its warm state across short AG-wait stalls."""
                for i in range(n):
                    nc.tensor.matmul(
                        deg_ps[:, 0, :],
                        ones[:, :],
                        at[:, (seed + i) % KB, 0:512],
                        start=True,
                        stop=True,
                    )

            # ---- layer 0 g ----
            g_sb = gsb_pool.tile([128, 8, R, H], BF16, name="g_sb", tag="gsb")
            for gi in range(2):
                g_out = make_g(0, gi, ftl)
                load_g(g_sb, gi, g_out, nc.sync)

            # ---- layers ----
            for l in range(L):
                yt_ps = psy.tile([H, P], F32, name="yt_ps", tag="ytps")
                ftl_next = ftl_pool.tile([H, P], F32, name="ftl", tag="ftl")
                g_sb_next = (
                    gsb_pool.tile([128, 8, R, H], BF16, name="g_sb", tag="gsb")
                    if l < L - 1
                    else None
                )

                for gi, (lo, hi) in enumerate(GROUPS):
                    for i, (k, r) in enumerate(KB_ORDER):
                        if gi == 0 and i == KB // 2 and N_FILL:
                            fillers(N_FILL, l * 7)
                        nc.tensor.matmul(
                            yt_ps[:, lo:hi],
                            g_sb[:, k, r, :],
                            at[:, r * 8 + k, lo:hi],
                            start=(i == 0),
                            stop=(i == KB - 1),
                        )
                        # splice the A-half g-for-next-layer into the middle
                        # of the hh1 stream (its epilogue runs on DVE/ACT
                        # just after hh0 stopped)
                        if i == 12 and gi == 1 and l < L - 1:
                            g_out = make_g(l + 1, 0, ftl_next)
                            load_g(g_sb_next, 0, g_out, nc.sync)
                    # epilogue for this pass's columns (runs under next pass);
                    # 2 chunks of 256 cols to shorten the trigger chain
                    for ci in range(2):
                        clo = lo + ci * 256
                        x1 = sp_pool.tile([H, 256], F32, name="x1", tag=f"sp_a{ci}")
                        nc.vector.tensor_tensor(
                            x1[:, :], yt_ps[:, clo : clo + 256],
                            isd_rep[:, clo : clo + 256], mybir.AluOpType.mult,
                        )
                        _softplus(
                            nc, sp_pool, ftl_next[:, clo : clo + 256], x1[:, :],
                            bsT_sb[:, l : l + 1], 256,
                        )
                # B-half g/AG (exposed tail, covered by next layer's A part
                # plus the fillers)
                if l < L - 1:
                    g_out = make_g(l + 1, 1, ftl_next)
                    load_g(g_sb_next, 1, g_out, nc.sync)
                    g_sb = g_sb_next
                ftl = ftl_next

            nc.sync.dma_start(out=out_ext[:, :], in_=ftl[:, :])

    nc.compile()
    return nc


def kernel(atom_pos, atom_emb, dist_adj, Ws, bs):
    global LAST_RESULT
    atom_pos = np.asarray(atom_pos, dtype=np.float32)
    atom_emb = np.asarray(atom_emb, dtype=np.float32)
    dist_adj = np.asarray(dist_adj, dtype=np.float32)
    Ws = np.asarray(Ws, dtype=np.float32)
    bs = np.asarray(bs, dtype=np.float32)

    feat = np.concatenate([atom_pos, atom_emb], axis=-1)  # [N, H]
    ws_bf = Ws.astype(ml_dtypes.bfloat16)
    bsT = np.ascontiguousarray(bs.T)  # [H, L]
    adj_bf = dist_adj.astype(ml_dtypes.bfloat16)

    if "nc" not in _NC_CACHE:
        _NC_CACHE["nc"] = build_nc()
    nc = _NC_CACHE["nc"]

    in_maps = []
    for c in range(R):
        rows = slice(c * P, (c + 1) * P)
        in_maps.append(
            {
                "adjT": np.ascontiguousarray(adj_bf[rows].T),  # [N, P] bf16
                "featT": np.ascontiguousarray(feat[rows].T),
                "ws": ws_bf,
                "bsT": bsT,
            }
        )

    trace = os.environ.get("K_TRACE", "0") == "1"
    kw = {}
    if trace:
        kw["trace_cores"] = list(range(R))
        kw["stitch_traces"] = os.environ.get("K_STITCH", "0") == "1"
    LAST_RESULT = run_bass_kernel_spmd(
        nc, in_maps, core_ids=list(range(R)), trace=trace, **kw
    )
    outs = [LAST_RESULT.results[c]["out"] for c in range(R)]  # each [H, P]
    return np.concatenate([o.T for o in outs], axis=0).astype(np.float32)


if __name__ == "__main__":
    rng = np.random.default_rng(0)
    out = kernel(
        rng.standard_normal((N, 3)).astype(np.float32),
        rng.standard_normal((N, 125)).astype(np.float32),
        rng.random((N, N), dtype=np.float32),
        (rng.standard_normal((L, H, H)) / np.sqrt(H)).astype(np.float32),
        np.zeros((L, H), np.float32),
    )
    print("out", out.shape, out.dtype, float(np.abs(out).mean()))


# revision 22
# speedup vs baseline: 1.0854x; 1.0854x over previous
"""AtomPosGNN distributed Trainium2 kernel (8 NeuronCores).

Reference computation (N=8192 nodes, H=128 features, L=4 layers):
    feat = concat(atom_pos, atom_emb)            # [N, 128]
    deg = dist_adj.sum(-1); isd = rsqrt(deg)
    for l in range(4):
        h = (feat * isd[:, None]) @ Ws[l]
        h = dist_adj @ h
        feat = softplus(h * isd[:, None] + bs[l])

Strategy (row shard, P=1024 rows per core, transpose-free):
  - Host passes each core its adj row-block ALREADY transposed and cast
    to bf16: adjT_c = dist_adj[rows_c, :].T  ([N, P] bf16, node index on
    partitions after tiling). No device-side transposes.
  - Prep: warm-AG trigger at t=0 on gpsimd (nothing else on that
    engine first), adjT streamed on the sync+scalar HWDGE queues; deg =
    ones-stationary matmul pass riding behind the DMA; isd via PE
    ones-broadcast + full-width reciprocal/sqrt.
  - Per layer the 1024 output columns run in 3 telescoped passes
    (512/256/256 cols). After each pass its epilogue + local g +
    AllGather (128/64/64KB) overlap the following passes; the next
    layer consumes kb tiles in matching group order (k0-3, k4-5, k6-7
    within each rank block, k-major) so only the small last AG is ever
    near the critical path.
  - adj is read from HBM exactly once; layers run entirely from SBUF.
"""

import os
import sys

for _p in ("/opt/trn_rl_repo",):
    if _p not in sys.path and os.path.isdir(_p):
        sys.path.insert(0, _p)

import numpy as np
import ml_dtypes

import concourse.bacc as bacc
import concourse.bass as bass
import concourse.mybir as mybir
import concourse.tile as tile
from concourse.bass_utils import run_bass_kernel_spmd

R = 8          # cores
N = 8192       # nodes
P = N // R     # local rows = 1024
H = 128        # hidden
L = 4          # layers
KB = N // 128  # 64 k-tiles
KBD = 4        # k-tiles per adj dma_start

F32 = mybir.dt.float32
BF16 = mybir.dt.bfloat16

WARM_AG = os.environ.get("K_WARM", "1") == "1"

LOG_A = float(np.log(2.0) / (1 << 23))
LOG_B = float(-np.log(2.0) * (127 + 0.0450466))

LAST_RESULT = None
_NC_CACHE = {}

# row/column halves: (lo, hi) in local node index
GROUPS = [(0, 512), (512, 1024)]
# kb consumption order: group-major, then k-major within group, then rank
KB_ORDER = []
for lo, hi in GROUPS:
    for k in range(lo // 128, hi // 128):
        for r in range(R):
            KB_ORDER.append((k, r))
N_FILL = int(os.environ.get("K_FILL", "0"))  # keep-warm MMs (Tile reorders them; off)
PE_DEG_KB = 16  # deg k-tiles on the PE; the rest accumulate on the DVE


def _softplus(nc, sp_pool, out_ap, in_ap, bias_ap, hw):
    """out = softplus(in + bias), composed (no Softplus/Ln table on HW)."""
    z0 = sp_pool.tile([H, hw], F32, name="z0", tag=f"sp_b{hw}")
    nc.scalar.activation(
        z0[:, :], in_ap, mybir.ActivationFunctionType.Exp, bias=bias_ap, scale=1.0
    )
    z = sp_pool.tile([H, hw], F32, name="z", tag=f"sp_c{hw}")
    nc.vector.tensor_scalar_add(z[:, :], z0[:, :], 1.0)
    y0 = sp_pool.tile([H, hw], F32, name="y0", tag=f"sp_d{hw}")
    nc.vector.tensor_scalar(
        y0[:, :], z[:, :].bitcast(mybir.dt.int32), LOG_A, LOG_B,
        mybir.AluOpType.mult, mybir.AluOpType.add,
    )
    w_e = sp_pool.tile([H, hw], F32, name="w_e", tag=f"sp_e{hw}")
    nc.scalar.activation(
        w_e[:, :], y0[:, :], mybir.ActivationFunctionType.Exp, scale=-1.0
    )
    t1 = sp_pool.tile([H, hw], F32, name="t1", tag=f"sp_f{hw}")
    nc.vector.tensor_tensor(t1[:, :], z[:, :], w_e[:, :], mybir.AluOpType.mult)
    nc.vector.tensor_scalar_add(t1[:, :], t1[:, :], -1.0)
    nc.vector.tensor_tensor(out_ap, t1[:, :], y0[:, :], mybir.AluOpType.add)


def build_nc():
    nc = bacc.Bacc("TRN2", target_bir_lowering=False, debug=False, num_devices=R)

    adjT_ext = nc.declare_dram_parameter("adjT", [N, P], BF16, isOutput=False)
    featT_ext = nc.declare_dram_parameter("featT", [H, P], F32, isOutput=False)
    ws_ext = nc.declare_dram_parameter("ws", [L, H, H], BF16, isOutput=False)
    bsT_ext = nc.declare_dram_parameter("bsT", [H, L], F32, isOutput=False)
    out_ext = nc.declare_dram_parameter("out", [H, P], F32, isOutput=True)

    with tile.TileContext(nc) as tc:
        with (
            tc.tile_pool(name="big", bufs=1) as big,
            tc.tile_pool(name="sb", bufs=1) as sb,
            tc.tile_pool(name="ftl", bufs=2) as ftl_pool,
            tc.tile_pool(name="pre", bufs=2) as pre_pool,
            tc.tile_pool(name="sp", bufs=1) as sp_pool,
            tc.tile_pool(name="gsb", bufs=2) as gsb_pool,
            tc.tile_pool(name="psd", bufs=1, space="PSUM") as psd,
            tc.tile_pool(name="psg", bufs=2, space="PSUM") as psg,
            tc.tile_pool(name="psy", bufs=2, space="PSUM") as psy,
            tc.tile_pool(name="dram", bufs=1, space="DRAM") as dram,
        ):
            # warm the collective path at the very start: the first
            # collective pays a large cold staging cost.
            if WARM_AG:
                warm_in = dram.tile([8, H], BF16, name="warm_in")
                warm_out = dram.tile([8 * R, H], BF16, addr_space="Shared", name="warm_out")
                nc.gpsimd.collective_compute(
                    "AllGather",
                    mybir.AluOpType.bypass,
                    replica_groups=[list(range(R))],
                    ins=[warm_in[:, :]],
                    outs=[warm_out[:, :]],
                )

            # ---- persistent SBUF ----
            at = big.tile([128, KB, P], BF16, name="at")  # adjT resident
            ones = sb.tile([128, 1], BF16, name="ones")
            nc.vector.memset(ones[:, :], 1.0)
            w_sb = sb.tile([128, L, H], BF16, name="w_sb")
            nc.sync.dma_start(
                out=w_sb[:, :, :], in_=ws_ext.rearrange("l k h -> k l h")
            )
            bsT_sb = sb.tile([H, L], F32, name="bsT_sb")
            nc.sync.dma_start(out=bsT_sb[:, :], in_=bsT_ext[:, :])
            ftl = ftl_pool.tile([H, P], F32, name="ftl", tag="ftl")
            nc.sync.dma_start(out=ftl[:, :], in_=featT_ext[:, :])

            # ---- prep: load adjT on the two HWDGE queues ----
            # small leading chunks so the deg pass can start early
            adjT_r = adjT_ext.rearrange("(kb p) r -> p kb r", p=128)
            qeng = [nc.sync, nc.scalar]
            chunks = []
            kb0 = 0
            while kb0 < KB:
                w = 2 if kb0 < 8 else KBD
                chunks.append((kb0, w))
                kb0 += w
            for ci, (kb0, w) in enumerate(chunks):
                qeng[ci % 2].dma_start(
                    out=at[:, kb0 : kb0 + w, :],
                    in_=adjT_r[:, kb0 : kb0 + w, :],
                )

            # deg[r] = sum_j adjT[j, r]: ones-stationary matmul pass
            deg_ps = psd.tile([1, 2, 512], F32, name="deg_ps")
            for kb in range(KB):
                for hh in range(2):
                    nc.tensor.matmul(
                        deg_ps[:, hh, :],
                        ones[:, :],
                        at[:, kb, hh * 512 : (hh + 1) * 512],
                        start=(kb == 0),
                        stop=(kb == KB - 1),
                    )
            # isd on one partition, then replicate to 128 partitions via a
            # DRAM bounce (stride-0 partition read); keeps PSUM banks free
            # for the double-buffered yt accumulator.
            isd_row = sb.tile([1, P], F32, name="isd_row")
            nc.vector.reciprocal(isd_row[0:1, :], deg_ps.rearrange("o h x -> o (h x)"))
            nc.scalar.sqrt(isd_row[0:1, :], isd_row[0:1, :])
            isd_dram = dram.tile([P], F32, name="isd_dram")
            nc.sync.dma_start(
                out=bass.AP(
                    tensor=isd_dram.tensor,
                    offset=isd_dram.offset,
                    ap=[[1, 1], [1, P]],
                ),
                in_=isd_row[0:1, :],
            )
            isd_rep = sb.tile([128, P], F32, name="isd_rep")
            nc.gpsimd.dma_start(
                out=isd_rep[:, :],
                in_=bass.AP(
                    tensor=isd_dram.tensor,
                    offset=isd_dram.offset,
                    ap=[[0, 128], [1, P]],
                ),
            )

            def make_g(l, gi, ftl_src):
                """Local g rows [lo, hi) -> DRAM, AllGather; returns AG out."""
                lo, hi = GROUPS[gi]
                kw = (hi - lo) // 128
                ftl_s = pre_pool.tile([H, hi - lo], BF16, name="ftl_s", tag=f"ftls{gi}")
                nc.vector.tensor_tensor(
                    ftl_s[:, :], ftl_src[:, lo:hi], isd_rep[:, lo:hi],
                    mybir.AluOpType.mult,
                )
                g_ps = psg.tile([128, kw, H], F32, name="g_ps", tag="gps")
                for nb in range(kw):
                    nc.tensor.matmul(
                        g_ps[:, nb, :],
                        ftl_s[:, nb * 128 : (nb + 1) * 128],
                        w_sb[:, l, :],
                        start=True,
                        stop=True,
                    )
                g_stage = pre_pool.tile(
                    [128, kw, H], BF16, name="g_stage", tag=f"gstage{gi}"
                )
                nc.vector.tensor_copy(g_stage[:, :, :], g_ps[:, :, :])
                g_in = dram.tile([hi - lo, H], BF16, name=f"g_in{l}_{gi}")
                nc.sync.dma_start(
                    out=g_in.rearrange("(k p) f -> p k f", p=128),
                    in_=g_stage[:, :, :],
                )
                g_out = dram.tile(
                    [R * (hi - lo), H], BF16, addr_space="Shared",
                    name=f"g_out{l}_{gi}",
                )
                nc.gpsimd.collective_compute(
                    "AllGather",
                    mybir.AluOpType.bypass,
                    replica_groups=[list(range(R))],
                    ins=[g_in[:, :]],
                    outs=[g_out[:, :]],
                )
                return g_out

            def load_g(g_sb, gi, g_out, eng):
                """One DMA per k-tile of the group into k-major g_sb."""
                lo, hi = GROUPS[gi]
                kw = (hi - lo) // 128
                g_out_r = g_out.rearrange("(r k p) f -> p k r f", p=128, k=kw)
                for k in range(kw):
                    eng.dma_start(
                        out=g_sb[:, lo // 128 + k, :, :],
                        in_=g_out_r[:, k, :, :],
                    )

            def fillers(n, seed):
                """Keep-warm MMs into the dead deg bank: hold the PE HAM
                at its warm state across short AG-wait stalls."""
                for i in range(n):
                    nc.tensor.matmul(
                        deg_ps[:, 0, :],
                        ones[:, :],
                        at[:, (seed + i) % KB, 0:512],
                        start=True,
                        stop=True,
                    )

            # ---- layer 0 g ----
            g_sb = gsb_pool.tile([128, 8, R, H], BF16, name="g_sb", tag="gsb")
            for gi in range(2):
                g_out = make_g(0, gi, ftl)
                load_g(g_sb, gi, g_out, nc.sync)

            # ---- layers ----
            for l in range(L):
                yt_ps = psy.tile([H, P], F32, name="yt_ps", tag="ytps")
                ftl_next = ftl_pool.tile([H, P], F32, name="ftl", tag="ftl")
                g_sb_next = (
                    gsb_pool.tile([128, 8, R, H], BF16, name="g_sb", tag="gsb")
                    if l < L - 1
                    else None
                )

                for gi, (lo, hi) in enumerate(GROUPS):
                    for i, (k, r) in enumerate(KB_ORDER):
                        if gi == 0 and i == KB // 2 and N_FILL:
                            fillers(N_FILL, l * 7)
                        nc.tensor.matmul(
                            yt_ps[:, lo:hi],
                            g_sb[:, k, r, :],
                            at[:, r * 8 + k, lo:hi],
                            start=(i == 0),
                            stop=(i == KB - 1),
                        )
                        # splice the A-half g-for-next-layer into the middle
                        # of the hh1 stream (its epilogue runs on DVE/ACT
                        # just after hh0 stopped)
                        if i == 12 and gi == 1 and l < L - 1:
                            g_out = make_g(l + 1, 0, ftl_next)
                            load_g(g_sb_next, 0, g_out, nc.sync)
                    # epilogue for this pass's columns (runs under next pass)
                    x1 = sp_pool.tile([H, 512], F32, name="x1", tag="sp_a")
                    nc.vector.tensor_tensor(
                        x1[:, :], yt_ps[:, lo:hi], isd_rep[:, lo:hi],
                        mybir.AluOpType.mult,
                    )
                    _softplus(
                        nc, sp_pool, ftl_next[:, lo:hi], x1[:, :],
                        bsT_sb[:, l : l + 1], 512,
                    )
                # B-half g/AG (exposed tail, covered by next layer's A part
                # plus the fillers)
                if l < L - 1:
                    g_out = make_g(l + 1, 1, ftl_next)
                    load_g(g_sb_next, 1, g_out, nc.sync)
                    g_sb = g_sb_next
                ftl = ftl_next

            nc.sync.dma_start(out=out_ext[:, :], in_=ftl[:, :])

    nc.compile()
    return nc


def kernel(atom_pos, atom_emb, dist_adj, Ws, bs):
    global LAST_RESULT
    atom_pos = np.asarray(atom_pos, dtype=np.float32)
    atom_emb = np.asarray(atom_emb, dtype=np.float32)
    dist_adj = np.asarray(dist_adj, dtype=np.float32)
    Ws = np.asarray(Ws, dtype=np.float32)
    bs = np.asarray(bs, dtype=np.float32)

    feat = np.concatenate([atom_pos, atom_emb], axis=-1)  # [N, H]
    ws_bf = Ws.astype(ml_dtypes.bfloat16)
    bsT = np.ascontiguousarray(bs.T)  # [H, L]
    adj_bf = dist_adj.astype(ml_dtypes.bfloat16)

    if "nc" not in _NC_CACHE:
        _NC_CACHE["nc"] = build_nc()
    nc = _NC_CACHE["nc"]

    in_maps = []
    for c in range(R):
        rows = slice(c * P, (c + 1) * P)
        in_maps.append(
            {
                "adjT": np.ascontiguousarray(adj_bf[rows].T),  # [N, P] bf16
                "featT": np.ascontiguousarray(feat[rows].T),
                "ws": ws_bf,
                "bsT": bsT,
            }
        )

    trace = os.environ.get("K_TRACE", "0") == "1"
    kw = {}
    if trace:
        kw["trace_cores"] = list(range(R))
        kw["stitch_traces"] = os.environ.get("K_STITCH", "0") == "1"
    LAST_RESULT = run_bass_kernel_spmd(
        nc, in_maps, core_ids=list(range(R)), trace=trace, **kw
    )
    outs = [LAST_RESULT.results[c]["out"] for c in range(R)]  # each [H, P]
    return np.concatenate([o.T for o in outs], axis=0).astype(np.float32)


if __name__ == "__main__":
    rng = np.random.default_rng(0)
    out = kernel(
        rng.standard_normal((N, 3)).astype(np.float32),
        rng.standard_normal((N, 125)).astype(np.float32),
        rng.random((N, N), dtype=np.float32),
        (rng.standard_normal((L, H, H)) / np.sqrt(H)).astype(np.float32),
        np.zeros((L, H), np.float32),
    )
    print("out", out.shape, out.dtype, float(np.abs(out).mean()))


# revision 25
# speedup vs baseline: 1.1208x; 1.0326x over previous
"""AtomPosGNN distributed Trainium2 kernel (8 NeuronCores).

Reference computation (N=8192 nodes, H=128 features, L=4 layers):
    feat = concat(atom_pos, atom_emb)            # [N, 128]
    deg = dist_adj.sum(-1); isd = rsqrt(deg)
    for l in range(4):
        h = (feat * isd[:, None]) @ Ws[l]
        h = dist_adj @ h
        feat = softplus(h * isd[:, None] + bs[l])

Strategy (row shard, P=1024 rows per core, transpose-free):
  - Host passes each core its adj row-block ALREADY transposed and cast
    to bf16: adjT_c = dist_adj[rows_c, :].T  ([N, P] bf16, node index on
    partitions after tiling). No device-side transposes.
  - Prep: warm-AG trigger at t=0 on gpsimd (nothing else on that
    engine first), adjT streamed on the sync+scalar HWDGE queues; deg =
    ones-stationary matmul pass riding behind the DMA; isd via PE
    ones-broadcast + full-width reciprocal/sqrt.
  - Per layer the 1024 output columns run in 3 telescoped passes
    (512/256/256 cols). After each pass its epilogue + local g +
    AllGather (128/64/64KB) overlap the following passes; the next
    layer consumes kb tiles in matching group order (k0-3, k4-5, k6-7
    within each rank block, k-major) so only the small last AG is ever
    near the critical path.
  - adj is read from HBM exactly once; layers run entirely from SBUF.
"""

import os
import sys

for _p in ("/opt/trn_rl_repo",):
    if _p not in sys.path and os.path.isdir(_p):
        sys.path.insert(0, _p)

import numpy as np
import ml_dtypes

import concourse.bacc as bacc
import concourse.bass as bass
import concourse.mybir as mybir
import concourse.tile as tile
from concourse.bass_utils import run_bass_kernel_spmd

R = 8          # cores
N = 8192       # nodes
P = N // R     # local rows = 1024
H = 128        # hidden
L = 4          # layers
KB = N // 128  # 64 k-tiles
KBD = 4        # k-tiles per adj dma_start

F32 = mybir.dt.float32
BF16 = mybir.dt.bfloat16

WARM_AG = os.environ.get("K_WARM", "1") == "1"

LOG_A = float(np.log(2.0) / (1 << 23))
LOG_B = float(-np.log(2.0) * (127 + 0.0450466))

LAST_RESULT = None
_NC_CACHE = {}

# row/column halves: (lo, hi) in local node index
GROUPS = [(0, 512), (512, 1024)]
# kb consumption order: group-major, then k-major within group, then rank
KB_ORDER = []
for lo, hi in GROUPS:
    for k in range(lo // 128, hi // 128):
        for r in range(R):
            KB_ORDER.append((k, r))
N_FILL = int(os.environ.get("K_FILL", "0"))  # keep-warm MMs (Tile reorders them; off)
PE_DEG_KB = 16  # deg k-tiles on the PE; the rest accumulate on the DVE


def _softplus(nc, sp_pool, out_ap, in_ap, bias_ap, hw):
    """out = softplus(in + bias), composed (no Softplus/Ln table on HW)."""
    z0 = sp_pool.tile([H, hw], F32, name="z0", tag=f"sp_b{hw}")
    nc.scalar.activation(
        z0[:, :], in_ap, mybir.ActivationFunctionType.Exp, bias=bias_ap, scale=1.0
    )
    z = sp_pool.tile([H, hw], F32, name="z", tag=f"sp_c{hw}")
    nc.vector.tensor_scalar_add(z[:, :], z0[:, :], 1.0)
    y0 = sp_pool.tile([H, hw], F32, name="y0", tag=f"sp_d{hw}")
    nc.vector.tensor_scalar(
        y0[:, :], z[:, :].bitcast(mybir.dt.int32), LOG_A, LOG_B,
        mybir.AluOpType.mult, mybir.AluOpType.add,
    )
    w_e = sp_pool.tile([H, hw], F32, name="w_e", tag=f"sp_e{hw}")
    nc.scalar.activation(
        w_e[:, :], y0[:, :], mybir.ActivationFunctionType.Exp, scale=-1.0
    )
    t1 = sp_pool.tile([H, hw], F32, name="t1", tag=f"sp_f{hw}")
    nc.vector.tensor_tensor(t1[:, :], z[:, :], w_e[:, :], mybir.AluOpType.mult)
    nc.vector.tensor_scalar_add(t1[:, :], t1[:, :], -1.0)
    nc.vector.tensor_tensor(out_ap, t1[:, :], y0[:, :], mybir.AluOpType.add)


def build_nc():
    nc = bacc.Bacc("TRN2", target_bir_lowering=False, debug=False, num_devices=R)

    adjT_ext = nc.declare_dram_parameter("adjT", [N, P], BF16, isOutput=False)
    featT_ext = nc.declare_dram_parameter("featT", [H, P], F32, isOutput=False)
    ws_ext = nc.declare_dram_parameter("ws", [L, H, H], BF16, isOutput=False)
    bsT_ext = nc.declare_dram_parameter("bsT", [H, L], F32, isOutput=False)
    out_ext = nc.declare_dram_parameter("out", [H, P], F32, isOutput=True)

    with tile.TileContext(nc) as tc:
        with (
            tc.tile_pool(name="big", bufs=1) as big,
            tc.tile_pool(name="sb", bufs=1) as sb,
            tc.tile_pool(name="ftl", bufs=2) as ftl_pool,
            tc.tile_pool(name="pre", bufs=2) as pre_pool,
            tc.tile_pool(name="sp", bufs=1) as sp_pool,
            tc.tile_pool(name="gsb", bufs=2) as gsb_pool,
            tc.tile_pool(name="psd", bufs=1, space="PSUM") as psd,
            tc.tile_pool(name="psg", bufs=2, space="PSUM") as psg,
            tc.tile_pool(name="psy", bufs=1, space="PSUM") as psy,
            tc.tile_pool(name="dram", bufs=1, space="DRAM") as dram,
        ):
            # warm the collective path at the very start: the first
            # collective pays a large cold staging cost.
            if WARM_AG:
                warm_in = dram.tile([8, H], BF16, name="warm_in")
                warm_out = dram.tile([8 * R, H], BF16, addr_space="Shared", name="warm_out")
                nc.gpsimd.collective_compute(
                    "AllGather",
                    mybir.AluOpType.bypass,
                    replica_groups=[list(range(R))],
                    ins=[warm_in[:, :]],
                    outs=[warm_out[:, :]],
                )

            # ---- persistent SBUF ----
            at = big.tile([128, KB, P], BF16, name="at")  # adjT resident
            ones = sb.tile([128, 1], BF16, name="ones")
            nc.vector.memset(ones[:, :], 1.0)
            w_sb = sb.tile([128, L, H], BF16, name="w_sb")
            nc.sync.dma_start(
                out=w_sb[:, :, :], in_=ws_ext.rearrange("l k h -> k l h")
            )
            bsT_sb = sb.tile([H, L], F32, name="bsT_sb")
            nc.sync.dma_start(out=bsT_sb[:, :], in_=bsT_ext[:, :])
            ftl = ftl_pool.tile([H, P], F32, name="ftl", tag="ftl")
            nc.sync.dma_start(out=ftl[:, :], in_=featT_ext[:, :])

            # ---- prep: load adjT on the two HWDGE queues ----
            # small leading chunks so the deg pass can start early
            adjT_r = adjT_ext.rearrange("(kb p) r -> p kb r", p=128)
            qeng = [nc.sync, nc.scalar]
            chunks = []
            kb0 = 0
            while kb0 < KB:
                w = 2 if kb0 < 8 else KBD
                chunks.append((kb0, w))
                kb0 += w
            for ci, (kb0, w) in enumerate(chunks):
                qeng[ci % 2].dma_start(
                    out=at[:, kb0 : kb0 + w, :],
                    in_=adjT_r[:, kb0 : kb0 + w, :],
                )

            # deg[r] = sum_j adjT[j, r]: ones-stationary matmul pass
            deg_ps = psd.tile([1, 2, 512], F32, name="deg_ps")
            for kb in range(KB):
                for hh in range(2):
                    nc.tensor.matmul(
                        deg_ps[:, hh, :],
                        ones[:, :],
                        at[:, kb, hh * 512 : (hh + 1) * 512],
                        start=(kb == 0),
                        stop=(kb == KB - 1),
                    )
            # isd: recip/sqrt on one partition, then broadcast to 128
            # partitions with a K=1 ones matmul (PE is free here)
            ones_row = sb.tile([1, 128], F32, name="ones_row")
            nc.vector.memset(ones_row[:, :], 1.0)
            isd_row = sb.tile([1, P], F32, name="isd_row")
            nc.vector.reciprocal(isd_row[0:1, :], deg_ps.rearrange("o h x -> o (h x)"))
            nc.scalar.sqrt(isd_row[0:1, :], isd_row[0:1, :])
            dbc_ps = psd.tile([128, 2, 512], F32, name="dbc_ps")
            for hh in range(2):
                nc.tensor.matmul(
                    dbc_ps[:, hh, :],
                    ones_row[:, :],
                    isd_row[0:1, hh * 512 : (hh + 1) * 512],
                    start=True,
                    stop=True,
                )
            isd_rep = sb.tile([128, P], F32, name="isd_rep")
            nc.vector.tensor_copy(isd_rep[:, :], dbc_ps.rearrange("p h x -> p (h x)"))

            def make_g(l, gi, ftl_src):
                """Local g rows [lo, hi) -> DRAM, AllGather; returns AG out.

                The whole chain runs under high scheduler priority so each
                engine services it before bulk MM/epilogue work; the g
                matmul writes bf16 PSUM which is DMAd to DRAM directly.
                """
                lo, hi = GROUPS[gi]
                kw = (hi - lo) // 128
                with tc.high_priority():
                    ftl_s = pre_pool.tile(
                        [H, hi - lo], BF16, name="ftl_s", tag=f"ftls{gi}"
                    )
                    nc.vector.tensor_tensor(
                        ftl_s[:, :], ftl_src[:, lo:hi], isd_rep[:, lo:hi],
                        mybir.AluOpType.mult,
                    )
                    g_ps = psg.tile([128, kw, H], F32, name="g_ps", tag="gps")
                    for nb in range(kw):
                        nc.tensor.matmul(
                            g_ps[:, nb, :],
                            ftl_s[:, nb * 128 : (nb + 1) * 128],
                            w_sb[:, l, :],
                            start=True,
                            stop=True,
                        )
                    g_stage = pre_pool.tile(
                        [128, kw, H], BF16, name="g_stage", tag=f"gstage{gi}"
                    )
                    nc.vector.tensor_copy(g_stage[:, :, :], g_ps[:, :, :])
                    g_in = dram.tile([hi - lo, H], BF16, name=f"g_in{l}_{gi}")
                    nc.sync.dma_start(
                        out=g_in.rearrange("(k p) f -> p k f", p=128),
                        in_=g_stage[:, :, :],
                    )
                    g_out = dram.tile(
                        [R * (hi - lo), H], BF16, addr_space="Shared",
                        name=f"g_out{l}_{gi}",
                    )
                    nc.gpsimd.collective_compute(
                        "AllGather",
                        mybir.AluOpType.bypass,
                        replica_groups=[list(range(R))],
                        ins=[g_in[:, :]],
                        outs=[g_out[:, :]],
                    )
                return g_out

            def load_g(g_sb, gi, g_out, eng):
                """One DMA per k-tile of the group into k-major g_sb."""
                lo, hi = GROUPS[gi]
                kw = (hi - lo) // 128
                g_out_r = g_out.rearrange("(r k p) f -> p k r f", p=128, k=kw)
                for k in range(kw):
                    eng.dma_start(
                        out=g_sb[:, lo // 128 + k, :, :],
                        in_=g_out_r[:, k, :, :],
                    )

            def fillers(n, seed):
                """Keep-warm MMs into the dead deg bank: hold the PE HAM
                at its warm state across short AG-wait stalls."""
                for i in range(n):
                    nc.tensor.matmul(
                        deg_ps[:, 0, :],
                        ones[:, :],
                        at[:, (seed + i) % KB, 0:512],
                        start=True,
                        stop=True,
                    )

            # ---- layer 0 g ----
            g_sb = gsb_pool.tile([128, 8, R, H], BF16, name="g_sb", tag="gsb")
            for gi in range(2):
                g_out = make_g(0, gi, ftl)
                load_g(g_sb, gi, g_out, nc.sync)

            # ---- layers ----
            for l in range(L):
                yt_ps = psy.tile([H, P], F32, name="yt_ps", tag="ytps")
                ftl_next = ftl_pool.tile([H, P], F32, name="ftl", tag="ftl")
                g_sb_next = (
                    gsb_pool.tile([128, 8, R, H], BF16, name="g_sb", tag="gsb")
                    if l < L - 1
                    else None
                )

                for gi, (lo, hi) in enumerate(GROUPS):
                    for i, (k, r) in enumerate(KB_ORDER):
                        if gi == 0 and i == KB // 2 and N_FILL:
                            fillers(N_FILL, l * 7)
                        nc.tensor.matmul(
                            yt_ps[:, lo:hi],
                            g_sb[:, k, r, :],
                            at[:, r * 8 + k, lo:hi],
                            start=(i == 0),
                            stop=(i == KB - 1),
                        )
                        # splice the A-half g-for-next-layer into the middle
                        # of the hh1 stream (its epilogue runs on DVE/ACT
                        # just after hh0 stopped)
                        if i == 12 and gi == 1 and l < L - 1:
                            g_out = make_g(l + 1, 0, ftl_next)
                            load_g(g_sb_next, 0, g_out, nc.sync)
                    # epilogue for this pass's columns (runs under next pass,
                    # high priority: it gates the AllGather chain)
                    with tc.high_priority():
                        x1 = sp_pool.tile([H, 512], F32, name="x1", tag="sp_a")
                        nc.vector.tensor_tensor(
                            x1[:, :], yt_ps[:, lo:hi], isd_rep[:, lo:hi],
                            mybir.AluOpType.mult,
                        )
                        _softplus(
                            nc, sp_pool, ftl_next[:, lo:hi], x1[:, :],
                            bsT_sb[:, l : l + 1], 512,
                        )
                # B-half g/AG (exposed tail, covered by next layer's A part
                # plus the fillers)
                if l < L - 1:
                    g_out = make_g(l + 1, 1, ftl_next)
                    load_g(g_sb_next, 1, g_out, nc.sync)
                    g_sb = g_sb_next
                ftl = ftl_next

            nc.sync.dma_start(out=out_ext[:, :], in_=ftl[:, :])

    nc.compile()
    return nc


def kernel(atom_pos, atom_emb, dist_adj, Ws, bs):
    global LAST_RESULT
    atom_pos = np.asarray(atom_pos, dtype=np.float32)
    atom_emb = np.asarray(atom_emb, dtype=np.float32)
    dist_adj = np.asarray(dist_adj, dtype=np.float32)
    Ws = np.asarray(Ws, dtype=np.float32)
    bs = np.asarray(bs, dtype=np.float32)

    feat = np.concatenate([atom_pos, atom_emb], axis=-1)  # [N, H]
    ws_bf = Ws.astype(ml_dtypes.bfloat16)
    bsT = np.ascontiguousarray(bs.T)  # [H, L]
    adj_bf = dist_adj.astype(ml_dtypes.bfloat16)

    if "nc" not in _NC_CACHE:
        _NC_CACHE["nc"] = build_nc()
    nc = _NC_CACHE["nc"]

    in_maps = []
    for c in range(R):
        rows = slice(c * P, (c + 1) * P)
        in_maps.append(
            {
                "adjT": np.ascontiguousarray(adj_bf[rows].T),  # [N, P] bf16
                "featT": np.ascontiguousarray(feat[rows].T),
                "ws": ws_bf,
                "bsT": bsT,
            }
        )

    trace = os.environ.get("K_TRACE", "0") == "1"
    kw = {}
    if trace:
        kw["trace_cores"] = list(range(R))
        kw["stitch_traces"] = os.environ.get("K_STITCH", "0") == "1"
    LAST_RESULT = run_bass_kernel_spmd(
        nc, in_maps, core_ids=list(range(R)), trace=trace, **kw
    )
    outs = [LAST_RESULT.results[c]["out"] for c in range(R)]  # each [H, P]
    return np.concatenate([o.T for o in outs], axis=0).astype(np.float32)


if __name__ == "__main__":
    rng = np.random.default_rng(0)
    out = kernel(
        rng.standard_normal((N, 3)).astype(np.float32),
        rng.standard_normal((N, 125)).astype(np.float32),
        rng.random((N, N), dtype=np.float32),
        (rng.standard_normal((L, H, H)) / np.sqrt(H)).astype(np.float32),
        np.zeros((L, H), np.float32),
    )
    print("out", out.shape, out.dtype, float(np.abs(out).mean()))
